# revision 58
# baseline (speedup 1.0000x reference)
"""Trainium2 Bass kernel: batched 64-digit base-10 addition (nn_Adder).

The reference RNN scan is carry-propagating decimal addition. The DVE
scan instruction is the only engine that can walk the carry recurrence,
and it runs at a fixed ~2.15 ns/element regardless of dtype — so the
kernel works in radix-10^4: each group of FOUR digits is one limb,
quartering the scan length per row (16 limbs instead of 64 digits).

Per core (pure data parallel across 8 cores, batch 524288 -> 65536 rows):

  * Inputs are uploaded as fp8e4 (digits 0-9 exact in e4m3): 4x less
    input HBM traffic than f32. Output leaves as one uint16 per FOUR
    digits (the raw scan state v = limb + 10^4*carry <= 19999, exact in
    u16): 8x less output traffic. The host decodes v % 10^4 into digit
    columns with numpy divmods.
  * Stage 1 (PE, fp8 DoubleRow perf mode): one DoubleRow matmul per
    source computes 10*d_even + d_odd for every digit pair — the
    weights [10I | I] pair with strided views of the even/odd digit
    positions (k-tile dim), and the MSB->LSB reversal is folded into
    the same access pattern. PSUM gets base-100 limbs M <= 198,
    LSB-first.
  * ACT drains M to SBUF as bf16 (integers <= 256 exact in bf16).
  * Stage 2 (PE, bf16): two accumulating matmuls with weights 100I / I
    over the odd/even base-100 limbs -> PSUM radix-10^4 limbs <= 19998.
  * DVE runs the whole carry chain in ONE scan per tile, reading PSUM:
    v_t = [10^4 <= v_{t-1}] + s_t. Row boundaries are killed by the
    data0 pattern operand (3e9 at each row's first limb). The scan
    writes the u16 OUTPUT tile directly - zero post-processing.
  * A burst of dummy matmuls right after the weight DMAs ramps the PE
    pstate (full clock needs ~3us of continuous execution) before the
    first data tile lands.
  * Small first/last tiles (G_LIST) shorten pipeline fill and drain.
  * GpSimd untouched (it would steal the DVE SBUF ports).

All intermediate values are small integers, exact in fp8/bf16/f32/u16 ->
bit-exact output after the host-side decode.
"""

import sys

sys.path.insert(0, "/opt/trn_rl_repo")

import numpy as np

BATCH = 524288
SEQ = 64
N_CORES = 8
B_LOC = BATCH // N_CORES

P = 128
LIMBS2 = SEQ // 2   # 32 base-100 limbs per row (stage-1)
LIMBS4 = SEQ // 4   # 16 base-10^4 limbs per row (stage-2 / scan / output)
G_LIST = [16] + [32] * 15 + [16]
G_MAX = max(G_LIST)
T = len(G_LIST)
FDM = G_MAX * SEQ       # padded digit cols (a/b tiles)
F2M = G_MAX * LIMBS2    # padded base-100 limb cols
F4M = G_MAX * LIMBS4    # padded base-10^4 limb cols
MW = 512                # matmul window = one PSUM bank of f32

IO_BUFS = (T + 1) // 2  # one buffer per (paired) load, all prefetched
WK_BUFS = 4

_nc_cache = {}


def _build_adder():
    from contextlib import ExitStack

    import concourse.bacc as bacc
    import concourse.mybir as mybir
    import concourse.tile as tile

    F32 = mybir.dt.float32
    BF16 = mybir.dt.bfloat16
    FP8 = mybir.dt.float8e4
    U16 = mybir.dt.uint16
    ALU = mybir.AluOpType
    DR = mybir.MatmulPerfMode.DoubleRow

    assert P * sum(G_LIST) == B_LOC

    nc = bacc.Bacc("TRN2", target_bir_lowering=False, debug=False)
    a_ext = nc.declare_dram_parameter("a", [B_LOC, SEQ], FP8, isOutput=False)
    b_ext = nc.declare_dram_parameter("b", [B_LOC, SEQ], FP8, isOutput=False)
    # [10I | I] fp8 pair-weights for DoubleRow stage 1
    edr_ext = nc.declare_dram_parameter("eyedr", [P, 2 * P], FP8,
                                        isOutput=False)
    # 100I and I in bf16 for stage 2
    e100_ext = nc.declare_dram_parameter("eye100", [P, P], BF16,
                                         isOutput=False)
    e1_ext = nc.declare_dram_parameter("eye1", [P, P], BF16, isOutput=False)
    o_ext = nc.declare_dram_parameter("out", [B_LOC, LIMBS4], U16,
                                      isOutput=True)

    with tile.TileContext(nc) as tc, ExitStack() as ctx:
        cpool = ctx.enter_context(tc.tile_pool(name="const", bufs=1))
        io = ctx.enter_context(tc.tile_pool(name="io", bufs=IO_BUFS))
        wk = ctx.enter_context(tc.tile_pool(name="wk", bufs=WK_BUFS))
        # one buffer per tile: output DMAs may drain late (they share
        # queues with the input stream) and must never stall the scans
        dpool = ctx.enter_context(tc.tile_pool(name="d",
                                               bufs=(T + 1) // 2))
        ps1 = ctx.enter_context(tc.tile_pool(name="ps1", bufs=3,
                                             space="PSUM"))
        ps2 = ctx.enter_context(tc.tile_pool(name="ps2", bufs=2,
                                             space="PSUM"))

        # One hardware DMA ring only sustains ~265 GB/s; the 16-engine
        # pool does 360 GB/s aggregate. Inputs are split across the
        # sync(SP) and scalar(ACT) rings. The a-stream is fully
        # prefetched; b-DMA issues are software-pipelined into the
        # compute loop so they don't sit in front of the ACT drains in
        # the scalar engine's in-order SEQ.
        # Tiles are DMA'd in paired "loads" (two tiles per transfer):
        # the framework recycles only ~9 DMA semaphores, so fewer,
        # larger DMAs both relieve that pool and amortize the ~0.8us
        # per-DMA queue overhead.
        LOOKAHEAD = 2       # b-LOADS prefetched ahead (= 4 tiles)
        # load 0 is a single small tile so the pipeline starts as soon
        # as possible; the rest are pairs
        LOADS = [(0, 1)] + [(i, min(i + 2, T)) for i in range(1, T, 2)]
        NL = len(LOADS)
        tl_map = {}
        for _L, (_t0, _t1) in enumerate(LOADS):
            for _t in range(_t0, _t1):
                tl_map[_t] = (_L, sum(G_LIST[_t0:_t]))
        row_base = [0]
        for G in G_LIST:
            row_base.append(row_base[-1] + P * G)

        io_loads = []
        b_lviews = []
        for L, (t0, t1) in enumerate(LOADS):
            GL = sum(G_LIST[t0:t1])
            R0, R1 = row_base[t0], row_base[t1]
            a_vt = a_ext[:][R0:R1].rearrange("(p g) e -> p (g e)", p=P)
            b_vt = b_ext[:][R0:R1].rearrange("(p g) e -> p (g e)", p=P)
            a_l = io.tile([P, GL * SEQ], FP8, tag="a", name=f"al_{L}",
                          padded_shape=[P, 2 * FDM])
            b_l = io.tile([P, GL * SEQ], FP8, tag="b", name=f"bl_{L}",
                          padded_shape=[P, 2 * FDM])
            io_loads.append((a_l, b_l))
            b_lviews.append(b_vt)
            if L == 0:
                # tiny weight tiles (96KB) FIRST: they unblock the PE
                # pstate warmup ~2us before the first data tile lands
                edr = cpool.tile([P, 2 * P], FP8)
                e100 = cpool.tile([P, P], BF16)
                e1 = cpool.tile([P, P], BF16)
                nc.sync.dma_start(out=edr[:], in_=edr_ext[:])
                nc.sync.dma_start(out=e100[:], in_=e100_ext[:])
                nc.sync.dma_start(out=e1[:], in_=e1_ext[:])
            nc.sync.dma_start(out=a_l[:], in_=a_vt)
            if L < LOOKAHEAD:
                nc.scalar.dma_start(out=b_l[:], in_=b_vt)
            if L == 0:
                pat = cpool.tile([P, F4M], F32)
                nc.vector.memset(pat[:], 10000.0)
                nc.vector.memset(pat[:, 0:F4M:LIMBS4], 3.0e9)

        def tile_load(t):
            """(load index, row-group offset of tile t within its load)"""
            return tl_map[t]

        edr3 = edr[:].rearrange("p (t m) -> p t m", t=2)

        # PE pstate warmup on the (early-landing) stage-1 weight tile:
        # the PE needs ~3us of continuous execution to reach full clock;
        # burn that on dummy matmuls before the first data tile is ready
        # (scratch lives in the ps2 pool; it is released long before the
        # second real stage-2 tile needs the buffer)
        warm = ps2.tile([P, P], F32, tag="ps2", name="warm",
                        padded_shape=[P, F4M])
        for _ in range(28):
            nc.tensor.matmul(warm[:], edr[:, 0:P], edr[:, 0:P],
                             start=True, stop=True)

        m_tiles = {}
        d_loads = {}

        def emit_stage1(t):
            """PE DoubleRow limb formation + ACT drain for tile t."""
            G = G_LIST[t]
            F2 = G * LIMBS2
            L, glo = tile_load(t)
            a_l, b_l = io_loads[L]
            a_t = a_l[:][:, glo * SEQ:(glo + G) * SEQ]
            b_t = b_l[:][:, glo * SEQ:(glo + G) * SEQ]
            # stage 1: base-100 limbs M = 10*(a+b)_hi + (a+b)_lo on PE.
            # rhs AP dims [p, t(k-tile), row, limb]: t picks the hi/lo
            # digit of each pair, limb stride -2 folds in the reversal.
            ps_t = ps1.tile([P, F2], F32, tag="ps1", name=f"ps1_{t}",
                            padded_shape=[P, F2M])
            A4 = a_t.rearrange("p (r m2 t) -> p t r m2",
                               t=2, m2=LIMBS2)[:, :, :, ::-1]
            B4 = b_t.rearrange("p (r m2 t) -> p t r m2",
                               t=2, m2=LIMBS2)[:, :, :, ::-1]
            W1 = min(MW, F2)
            RW1 = W1 // LIMBS2
            for h in range(F2 // W1):
                win = ps_t[:, h * W1:(h + 1) * W1]
                rs = slice(h * RW1, (h + 1) * RW1)
                nc.tensor.matmul(win, edr3, A4[:, :, rs], start=True,
                                 stop=False, perf_mode=DR)
                nc.tensor.matmul(win, edr3, B4[:, :, rs], start=False,
                                 stop=True, perf_mode=DR)
            # software-pipelined b-stream (strict load order: FIFO ring);
            # issued ahead of the drain so the SEQ fires it immediately
            L1, _ = tile_load(t)
            if t == LOADS[L1][0] and L1 + LOOKAHEAD < NL:
                nl = L1 + LOOKAHEAD
                nc.scalar.dma_start(out=io_loads[nl][1][:],
                                    in_=b_lviews[nl])
            # ACT drains M to SBUF bf16 (exact, M <= 198)
            m_t = wk.tile([P, F2], BF16, tag="m", name=f"m_{t}",
                          padded_shape=[P, F2M])
            nc.scalar.activation(m_t[:], ps_t[:],
                                 mybir.ActivationFunctionType.Copy)
            m_tiles[t] = m_t

        def emit_stage2(t):
            """PE radix-10^4 combine + DVE scan + out DMA for tile t."""
            G = G_LIST[t]
            F4 = G * LIMBS4
            L, glo = tile_load(t)
            t0, t1 = LOADS[L]
            m_t = m_tiles.pop(t)
            # stage 2: radix-10^4 limbs L = 100*M_odd + M_even on PE
            ps4_t = ps2.tile([P, F4], F32, tag="ps2", name=f"ps2_{t}",
                             padded_shape=[P, F4M])
            M3 = m_t[:].rearrange("p (r q t) -> p r q t", t=2, q=LIMBS4)
            W2 = min(MW, F4)
            RW2 = W2 // LIMBS4
            for h in range(F4 // W2):
                win = ps4_t[:, h * W2:(h + 1) * W2]
                rs = slice(h * RW2, (h + 1) * RW2)
                nc.tensor.matmul(win, e100[:], M3[:, rs, :, 1], start=True,
                                 stop=False)
                nc.tensor.matmul(win, e1[:], M3[:, rs, :, 0], start=False,
                                 stop=True)
            # whole carry chain: v_t = [10^4 <= v_{t-1}] + s_t, written
            # straight into this load's u16 output tile (v <= 19999)
            if t == t0:
                GL = sum(G_LIST[t0:t1])
                d_loads[L] = dpool.tile([P, GL * LIMBS4], U16, tag="d",
                                        name=f"d_{L}",
                                        padded_shape=[P, 2 * F4M])
            d_l = d_loads[L]
            nc.vector.tensor_tensor_scan(
                out=d_l[:][:, glo * LIMBS4:(glo + G) * LIMBS4],
                data0=pat[:, 0:F4], data1=ps4_t[:],
                initial=0.0, op0=ALU.is_le, op1=ALU.add)
            # one output DMA per load, after its last scan; outs go on
            # the sync ring (fully prefetched there, so a waiting out
            # blocks nothing)
            if t == t1 - 1:
                R0, R1 = row_base[t0], row_base[t1]
                o_vl = o_ext[:][R0:R1].rearrange("(p g) e -> p (g e)", p=P)
                nc.sync.dma_start(out=o_vl, in_=d_l[:])

        # PE stream software-pipelined one tile ahead: while stage2_t
        # sits in the in-order PE queue waiting for tile t's ACT drain,
        # the PE executes tile t+1's stage-1 matmuls instead of idling.
        emit_stage1(0)
        for t in range(T):
            if t + 1 < T:
                emit_stage1(t + 1)
            emit_stage2(t)

    nc.finalize()
    return nc


def _host_inputs(a, b):
    """Cast digit arrays to fp8 (exact for 0..9) and build per-core maps."""
    import ml_dtypes

    fp8 = ml_dtypes.float8_e4m3
    bf16 = ml_dtypes.bfloat16
    a8 = np.ascontiguousarray(np.asarray(a, dtype=np.float32)).astype(fp8)
    b8 = np.ascontiguousarray(np.asarray(b, dtype=np.float32)).astype(fp8)
    eye = np.eye(P, dtype=np.float32)
    eyedr = np.concatenate([10.0 * eye, eye], axis=1).astype(fp8)
    eye100 = (100.0 * eye).astype(bf16)
    eye1 = eye.astype(bf16)
    return [
        {"a": a8[i * B_LOC:(i + 1) * B_LOC],
         "b": b8[i * B_LOC:(i + 1) * B_LOC],
         "eyedr": eyedr, "eye100": eye100, "eye1": eye1}
        for i in range(N_CORES)
    ]


def _host_decode(results):
    """Concat per-core raw scan words (v = limb + 10^4*carry, LSB-first
    limb order) and decode into f32 digit columns."""
    raw = np.concatenate(
        [results[i]["out"] for i in range(N_CORES)], axis=0)  # (B, 16) u16
    v = (raw[:, ::-1] % 10000).astype(np.int32)
    out = np.empty((BATCH, SEQ), dtype=np.float32)
    q, out_3 = np.divmod(v, 10)
    q, out_2 = np.divmod(q, 10)
    out_0, out_1 = np.divmod(q, 10)
    out[:, 0::4] = out_0
    out[:, 1::4] = out_1
    out[:, 2::4] = out_2
    out[:, 3::4] = out_3
    return out


def kernel(a, b, weight_ih=None, weight_hh=None, bias_ih=None, bias_hh=None):
    """Full-batch digit adder. The RNN weights are the fixed carry-add
    weights baked into the module; the kernel implements that function
    directly, so they are accepted and unused."""
    from concourse.bass_utils import run_bass_kernel_spmd

    assert np.asarray(a).shape == (BATCH, SEQ)
    assert np.asarray(b).shape == (BATCH, SEQ)

    if "nc" not in _nc_cache:
        _nc_cache["nc"] = _build_adder()
    nc = _nc_cache["nc"]

    res = run_bass_kernel_spmd(nc, _host_inputs(a, b),
                               core_ids=list(range(N_CORES)))
    return _host_decode(res.results)


if __name__ == "__main__":
    rng = np.random.default_rng(0)
    a = rng.integers(0, 10, (BATCH, SEQ)).astype(np.float32)
    b = rng.integers(0, 10, (BATCH, SEQ)).astype(np.float32)
    out = kernel(a, b)
    # host reference
    c = np.zeros(BATCH, np.float32)
    exp = np.zeros_like(a)
    for e in range(SEQ - 1, -1, -1):
        s = a[:, e] + b[:, e] + c
        c = (s >= 10).astype(np.float32)
        exp[:, e] = s - 10 * c
    print("max abs err:", np.abs(out - exp).max())


# revision 59
# speedup vs baseline: 1.0232x; 1.0232x over previous
"""Trainium2 Bass kernel: batched 64-digit base-10 addition (nn_Adder).

The reference RNN scan is carry-propagating decimal addition. The DVE
scan instruction is the only engine that can walk the carry recurrence,
and it runs at a fixed ~2.15 ns/element regardless of dtype — so the
kernel works in radix-10^4: each group of FOUR digits is one limb,
quartering the scan length per row (16 limbs instead of 64 digits).

Per core (pure data parallel across 8 cores, batch 524288 -> 65536 rows):

  * Inputs are uploaded as fp8e4 (digits 0-9 exact in e4m3): 4x less
    input HBM traffic than f32. Output leaves as one uint16 per FOUR
    digits (the raw scan state v = limb + 10^4*carry <= 19999, exact in
    u16): 8x less output traffic. The host decodes v % 10^4 into digit
    columns with numpy divmods.
  * Stage 1 (PE, fp8 DoubleRow perf mode): one DoubleRow matmul per
    source computes 10*d_even + d_odd for every digit pair — the
    weights [10I | I] pair with strided views of the even/odd digit
    positions (k-tile dim), and the MSB->LSB reversal is folded into
    the same access pattern. PSUM gets base-100 limbs M <= 198,
    LSB-first.
  * ACT drains M to SBUF as bf16 (integers <= 256 exact in bf16).
  * Stage 2 (PE, bf16): two accumulating matmuls with weights 100I / I
    over the odd/even base-100 limbs -> PSUM radix-10^4 limbs <= 19998.
  * DVE runs the whole carry chain in ONE scan per tile, reading PSUM:
    v_t = [10^4 <= v_{t-1}] + s_t. Row boundaries are killed by the
    data0 pattern operand (3e9 at each row's first limb). The scan
    writes the u16 OUTPUT tile directly - zero post-processing.
  * A burst of dummy matmuls right after the weight DMAs ramps the PE
    pstate (full clock needs ~3us of continuous execution) before the
    first data tile lands.
  * Small first/last tiles (G_LIST) shorten pipeline fill and drain.
  * GpSimd untouched (it would steal the DVE SBUF ports).

All intermediate values are small integers, exact in fp8/bf16/f32/u16 ->
bit-exact output after the host-side decode.
"""

import sys

sys.path.insert(0, "/opt/trn_rl_repo")

import numpy as np

BATCH = 524288
SEQ = 64
N_CORES = 8
B_LOC = BATCH // N_CORES

P = 128
LIMBS2 = SEQ // 2   # 32 base-100 limbs per row (stage-1)
LIMBS4 = SEQ // 4   # 16 base-10^4 limbs per row (stage-2 / scan / output)
G_LIST = [16] + [32] * 15 + [16]
G_MAX = max(G_LIST)
T = len(G_LIST)
FDM = G_MAX * SEQ       # padded digit cols (a/b tiles)
F2M = G_MAX * LIMBS2    # padded base-100 limb cols
F4M = G_MAX * LIMBS4    # padded base-10^4 limb cols
MW = 512                # matmul window = one PSUM bank of f32

IO_BUFS = (T + 1) // 2  # one buffer per (paired) load, all prefetched
WK_BUFS = 4

_nc_cache = {}


def _build_adder():
    from contextlib import ExitStack

    import concourse.bacc as bacc
    import concourse.mybir as mybir
    import concourse.tile as tile

    F32 = mybir.dt.float32
    BF16 = mybir.dt.bfloat16
    FP8 = mybir.dt.float8e4
    U16 = mybir.dt.uint16
    ALU = mybir.AluOpType
    DR = mybir.MatmulPerfMode.DoubleRow

    assert P * sum(G_LIST) == B_LOC

    nc = bacc.Bacc("TRN2", target_bir_lowering=False, debug=False)
    a_ext = nc.declare_dram_parameter("a", [B_LOC, SEQ], FP8, isOutput=False)
    b_ext = nc.declare_dram_parameter("b", [B_LOC, SEQ], FP8, isOutput=False)
    # [10I | I] fp8 pair-weights for DoubleRow stage 1
    edr_ext = nc.declare_dram_parameter("eyedr", [P, 2 * P], FP8,
                                        isOutput=False)
    # 100I and I in bf16 for stage 2
    e100_ext = nc.declare_dram_parameter("eye100", [P, P], BF16,
                                         isOutput=False)
    e1_ext = nc.declare_dram_parameter("eye1", [P, P], BF16, isOutput=False)
    o_ext = nc.declare_dram_parameter("out", [B_LOC, LIMBS4], U16,
                                      isOutput=True)

    with tile.TileContext(nc) as tc, ExitStack() as ctx:
        cpool = ctx.enter_context(tc.tile_pool(name="const", bufs=1))
        io = ctx.enter_context(tc.tile_pool(name="io", bufs=IO_BUFS))
        wk = ctx.enter_context(tc.tile_pool(name="wk", bufs=WK_BUFS))
        # one buffer per tile: output DMAs may drain late (they share
        # queues with the input stream) and must never stall the scans
        dpool = ctx.enter_context(tc.tile_pool(name="d",
                                               bufs=(T + 1) // 2))
        ps1 = ctx.enter_context(tc.tile_pool(name="ps1", bufs=3,
                                             space="PSUM"))
        ps2 = ctx.enter_context(tc.tile_pool(name="ps2", bufs=2,
                                             space="PSUM"))

        # One hardware DMA ring only sustains ~265 GB/s; the 16-engine
        # pool does 360 GB/s aggregate. Inputs are split across the
        # sync(SP) and scalar(ACT) rings. The a-stream is fully
        # prefetched; b-DMA issues are software-pipelined into the
        # compute loop so they don't sit in front of the ACT drains in
        # the scalar engine's in-order SEQ.
        # Tiles are DMA'd in paired "loads" (two tiles per transfer):
        # the framework recycles only ~9 DMA semaphores, so fewer,
        # larger DMAs both relieve that pool and amortize the ~0.8us
        # per-DMA queue overhead.
        LOOKAHEAD = 2       # b-LOADS prefetched ahead (= 4 tiles)
        LOADS = [(i, min(i + 2, T)) for i in range(0, T, 2)]
        NL = len(LOADS)
        tl_map = {}
        for _L, (_t0, _t1) in enumerate(LOADS):
            for _t in range(_t0, _t1):
                tl_map[_t] = (_L, sum(G_LIST[_t0:_t]))
        row_base = [0]
        for G in G_LIST:
            row_base.append(row_base[-1] + P * G)

        io_loads = []
        b_lviews = []
        for L, (t0, t1) in enumerate(LOADS):
            GL = sum(G_LIST[t0:t1])
            R0, R1 = row_base[t0], row_base[t1]
            a_vt = a_ext[:][R0:R1].rearrange("(p g) e -> p (g e)", p=P)
            b_vt = b_ext[:][R0:R1].rearrange("(p g) e -> p (g e)", p=P)
            a_l = io.tile([P, GL * SEQ], FP8, tag="a", name=f"al_{L}",
                          padded_shape=[P, 2 * FDM])
            b_l = io.tile([P, GL * SEQ], FP8, tag="b", name=f"bl_{L}",
                          padded_shape=[P, 2 * FDM])
            io_loads.append((a_l, b_l))
            b_lviews.append(b_vt)
            if L == 0:
                # tiny weight tiles (96KB) FIRST: they unblock the PE
                # pstate warmup ~2us before the first data tile lands
                edr = cpool.tile([P, 2 * P], FP8)
                e100 = cpool.tile([P, P], BF16)
                e1 = cpool.tile([P, P], BF16)
                nc.sync.dma_start(out=edr[:], in_=edr_ext[:])
                nc.sync.dma_start(out=e100[:], in_=e100_ext[:])
                nc.sync.dma_start(out=e1[:], in_=e1_ext[:])
            nc.sync.dma_start(out=a_l[:], in_=a_vt)
            if L < LOOKAHEAD:
                nc.scalar.dma_start(out=b_l[:], in_=b_vt)
            if L == 0:
                pat = cpool.tile([P, F4M], F32)
                nc.vector.memset(pat[:], 10000.0)
                nc.vector.memset(pat[:, 0:F4M:LIMBS4], 3.0e9)

        def tile_load(t):
            """(load index, row-group offset of tile t within its load)"""
            return tl_map[t]

        edr3 = edr[:].rearrange("p (t m) -> p t m", t=2)

        # PE pstate warmup on the (early-landing) stage-1 weight tile:
        # the PE needs ~3us of continuous execution to reach full clock;
        # burn that on dummy matmuls before the first data tile is ready
        # (scratch lives in the ps2 pool; it is released long before the
        # second real stage-2 tile needs the buffer)
        warm = ps2.tile([P, P], F32, tag="ps2", name="warm",
                        padded_shape=[P, F4M])
        for _ in range(28):
            nc.tensor.matmul(warm[:], edr[:, 0:P], edr[:, 0:P],
                             start=True, stop=True)

        m_tiles = {}
        d_loads = {}

        def emit_stage1(t):
            """PE DoubleRow limb formation + ACT drain for tile t."""
            G = G_LIST[t]
            F2 = G * LIMBS2
            L, glo = tile_load(t)
            a_l, b_l = io_loads[L]
            a_t = a_l[:][:, glo * SEQ:(glo + G) * SEQ]
            b_t = b_l[:][:, glo * SEQ:(glo + G) * SEQ]
            # stage 1: base-100 limbs M = 10*(a+b)_hi + (a+b)_lo on PE.
            # rhs AP dims [p, t(k-tile), row, limb]: t picks the hi/lo
            # digit of each pair, limb stride -2 folds in the reversal.
            ps_t = ps1.tile([P, F2], F32, tag="ps1", name=f"ps1_{t}",
                            padded_shape=[P, F2M])
            A4 = a_t.rearrange("p (r m2 t) -> p t r m2",
                               t=2, m2=LIMBS2)[:, :, :, ::-1]
            B4 = b_t.rearrange("p (r m2 t) -> p t r m2",
                               t=2, m2=LIMBS2)[:, :, :, ::-1]
            W1 = min(MW, F2)
            RW1 = W1 // LIMBS2
            for h in range(F2 // W1):
                win = ps_t[:, h * W1:(h + 1) * W1]
                rs = slice(h * RW1, (h + 1) * RW1)
                nc.tensor.matmul(win, edr3, A4[:, :, rs], start=True,
                                 stop=False, perf_mode=DR)
                nc.tensor.matmul(win, edr3, B4[:, :, rs], start=False,
                                 stop=True, perf_mode=DR)
            # software-pipelined b-stream (strict load order: FIFO ring);
            # issued ahead of the drain so the SEQ fires it immediately
            L1, _ = tile_load(t)
            if t == LOADS[L1][0] and L1 + LOOKAHEAD < NL:
                nl = L1 + LOOKAHEAD
                nc.scalar.dma_start(out=io_loads[nl][1][:],
                                    in_=b_lviews[nl])
            # ACT drains M to SBUF bf16 (exact, M <= 198)
            m_t = wk.tile([P, F2], BF16, tag="m", name=f"m_{t}",
                          padded_shape=[P, F2M])
            nc.scalar.activation(m_t[:], ps_t[:],
                                 mybir.ActivationFunctionType.Copy)
            m_tiles[t] = m_t

        def emit_stage2(t):
            """PE radix-10^4 combine + DVE scan + out DMA for tile t."""
            G = G_LIST[t]
            F4 = G * LIMBS4
            L, glo = tile_load(t)
            t0, t1 = LOADS[L]
            m_t = m_tiles.pop(t)
            # stage 2: radix-10^4 limbs L = 100*M_odd + M_even on PE
            ps4_t = ps2.tile([P, F4], F32, tag="ps2", name=f"ps2_{t}",
                             padded_shape=[P, F4M])
            M3 = m_t[:].rearrange("p (r q t) -> p r q t", t=2, q=LIMBS4)
            W2 = min(MW, F4)
            RW2 = W2 // LIMBS4
            for h in range(F4 // W2):
                win = ps4_t[:, h * W2:(h + 1) * W2]
                rs = slice(h * RW2, (h + 1) * RW2)
                nc.tensor.matmul(win, e100[:], M3[:, rs, :, 1], start=True,
                                 stop=False)
                nc.tensor.matmul(win, e1[:], M3[:, rs, :, 0], start=False,
                                 stop=True)
            # whole carry chain: v_t = [10^4 <= v_{t-1}] + s_t, written
            # straight into this load's u16 output tile (v <= 19999)
            if t == t0:
                GL = sum(G_LIST[t0:t1])
                d_loads[L] = dpool.tile([P, GL * LIMBS4], U16, tag="d",
                                        name=f"d_{L}",
                                        padded_shape=[P, 2 * F4M])
            d_l = d_loads[L]
            nc.vector.tensor_tensor_scan(
                out=d_l[:][:, glo * LIMBS4:(glo + G) * LIMBS4],
                data0=pat[:, 0:F4], data1=ps4_t[:],
                initial=0.0, op0=ALU.is_le, op1=ALU.add)
            # one output DMA per load, after its last scan; outs go on
            # the sync ring (fully prefetched there, so a waiting out
            # blocks nothing)
            if t == t1 - 1:
                R0, R1 = row_base[t0], row_base[t1]
                o_vl = o_ext[:][R0:R1].rearrange("(p g) e -> p (g e)", p=P)
                nc.sync.dma_start(out=o_vl, in_=d_l[:])

        # PE stream software-pipelined one tile ahead: while stage2_t
        # sits in the in-order PE queue waiting for tile t's ACT drain,
        # the PE executes tile t+1's stage-1 matmuls instead of idling.
        emit_stage1(0)
        for t in range(T):
            if t + 1 < T:
                emit_stage1(t + 1)
            emit_stage2(t)

    nc.finalize()
    return nc


def _host_inputs(a, b):
    """Cast digit arrays to fp8 (exact for 0..9) and build per-core maps."""
    import ml_dtypes

    fp8 = ml_dtypes.float8_e4m3
    bf16 = ml_dtypes.bfloat16
    a8 = np.ascontiguousarray(np.asarray(a, dtype=np.float32)).astype(fp8)
    b8 = np.ascontiguousarray(np.asarray(b, dtype=np.float32)).astype(fp8)
    eye = np.eye(P, dtype=np.float32)
    eyedr = np.concatenate([10.0 * eye, eye], axis=1).astype(fp8)
    eye100 = (100.0 * eye).astype(bf16)
    eye1 = eye.astype(bf16)
    return [
        {"a": a8[i * B_LOC:(i + 1) * B_LOC],
         "b": b8[i * B_LOC:(i + 1) * B_LOC],
         "eyedr": eyedr, "eye100": eye100, "eye1": eye1}
        for i in range(N_CORES)
    ]


def _host_decode(results):
    """Concat per-core raw scan words (v = limb + 10^4*carry, LSB-first
    limb order) and decode into f32 digit columns."""
    raw = np.concatenate(
        [results[i]["out"] for i in range(N_CORES)], axis=0)  # (B, 16) u16
    v = (raw[:, ::-1] % 10000).astype(np.int32)
    out = np.empty((BATCH, SEQ), dtype=np.float32)
    q, out_3 = np.divmod(v, 10)
    q, out_2 = np.divmod(q, 10)
    out_0, out_1 = np.divmod(q, 10)
    out[:, 0::4] = out_0
    out[:, 1::4] = out_1
    out[:, 2::4] = out_2
    out[:, 3::4] = out_3
    return out


def kernel(a, b, weight_ih=None, weight_hh=None, bias_ih=None, bias_hh=None):
    """Full-batch digit adder. The RNN weights are the fixed carry-add
    weights baked into the module; the kernel implements that function
    directly, so they are accepted and unused."""
    from concourse.bass_utils import run_bass_kernel_spmd

    assert np.asarray(a).shape == (BATCH, SEQ)
    assert np.asarray(b).shape == (BATCH, SEQ)

    if "nc" not in _nc_cache:
        _nc_cache["nc"] = _build_adder()
    nc = _nc_cache["nc"]

    res = run_bass_kernel_spmd(nc, _host_inputs(a, b),
                               core_ids=list(range(N_CORES)))
    return _host_decode(res.results)


if __name__ == "__main__":
    rng = np.random.default_rng(0)
    a = rng.integers(0, 10, (BATCH, SEQ)).astype(np.float32)
    b = rng.integers(0, 10, (BATCH, SEQ)).astype(np.float32)
    out = kernel(a, b)
    # host reference
    c = np.zeros(BATCH, np.float32)
    exp = np.zeros_like(a)
    for e in range(SEQ - 1, -1, -1):
        s = a[:, e] + b[:, e] + c
        c = (s >= 10).astype(np.float32)
        exp[:, e] = s - 10 * c
    print("max abs err:", np.abs(out - exp).max())
